# revision 17
# baseline (speedup 1.0000x reference)
"""Trainium2 Bass kernel for nn_LocalFeatureEncoder (fp16, blkdiag 2-strip).

Computes, for B=8 batches on 8 NeuronCores (batch b -> core b):
    g      = concat(shape_code, structure_code, pose_code)      # (B, 128)
    local  = einsum('kfz,bz->bkf', W, g) + bias                 # (B, 24, 64)
    out    = einsum('btk,bkf->btf', lbs_weights, local)         # (B, 32768, 64)

All device data is fp16 (f32 PSUM accumulation). HBM per core:
1.57 MB lbs + 0.45 MB consts in, 4.19 MB out.

Device program per core:
  Stage 1: 12 matmuls (Wt 128-col blocks stationary, g moving, N=1) ->
    local in column layout [128, 12]; DVE bias-add (fp16 cast);
    PE-transpose -> [12, 128]; DRAM roundtrip (on the gpsimd SWDGE queue,
    so it never queues behind the big input streams) scatters
    blkdiag(local, local) [48, 128] stationaries to partition bases 0 / 64.
  Main: lbs host-pretransposed to k-on-partition layout: strip A rows
    0-47 = (t 0..8191 | t 8192..16383) as 2x24 rows, strip B rows 64-111
    same for t 16384..32767. 16 psum tiles [128, 1024] (2 banks), 2
    matmuls N=512 each; strips A/B run on disjoint PE row groups with
    resident weights. PSUM evacuation (f32 -> fp16) alternates
    Vector/Scalar into a [128, 16384] staging buffer; each cast's 1024
    columns go out immediately as a 256 KB DMA on the sync ring.

Host: pre-casts/transposes inputs; un-transposes device output.
"""

import os
from contextlib import ExitStack

import numpy as np

import concourse.bass as bass
import concourse.bacc as bacc
import concourse.tile as tile
from concourse import mybir
from concourse import bass_utils

B, T, K, Z, F = 8, 32768, 24, 128, 64
KF = K * F              # 1536
TH = T // 4             # 8192 columns per strip
NWT = KF // Z           # 12 stage-1 weight blocks

_built = {}


def _build():
    if "nc" in _built:
        return _built["nc"]

    f32 = mybir.dt.float32
    f16 = mybir.dt.float16
    nc = bacc.Bacc("TRN2", target_bir_lowering=False, debug=False)

    lbs_d = nc.dram_tensor("lbs", (4 * K, TH), f16, kind="ExternalInput")
    wt_d = nc.dram_tensor("wt", (Z, KF), f16, kind="ExternalInput")
    g_d = nc.dram_tensor("g", (Z, 1), f16, kind="ExternalInput")
    biasc_d = nc.dram_tensor("biasc", (Z, NWT), f16, kind="ExternalInput")
    ident_d = nc.dram_tensor("ident", (Z, Z), f16, kind="ExternalInput")
    out_d = nc.dram_tensor("out", (128, T // 2), f16, kind="ExternalOutput")

    with tile.TileContext(nc) as tc, ExitStack() as ctx:
        const = ctx.enter_context(tc.tile_pool(name="const", bufs=1))
        big = ctx.enter_context(tc.tile_pool(name="big", bufs=1))
        psO = ctx.enter_context(
            tc.tile_pool(name="psO", bufs=2, space=bass.MemorySpace.PSUM)
        )
        dram = ctx.enter_context(
            tc.tile_pool(name="dram", bufs=1, space=bass.MemorySpace.DRAM)
        )

        # ---- small const loads first so stage 1 starts early ----
        wt_sb = const.tile([Z, KF], f16)
        hk = KF // 4
        nc.sync.dma_start(wt_sb[:, 0:hk], wt_d.ap()[:, 0:hk])
        nc.scalar.dma_start(wt_sb[:, hk:2 * hk], wt_d.ap()[:, hk:2 * hk])
        nc.sync.dma_start(wt_sb[:, 2 * hk:3 * hk], wt_d.ap()[:, 2 * hk:3 * hk])
        nc.scalar.dma_start(wt_sb[:, 3 * hk:KF], wt_d.ap()[:, 3 * hk:KF])
        g_sb = const.tile([Z, 1], f16)
        nc.sync.dma_start(g_sb[:], g_d.ap())
        biasc_sb = const.tile([Z, NWT], f16)
        nc.scalar.dma_start(biasc_sb[:], biasc_d.ap())
        ident = const.tile([Z, Z], f16)
        nc.sync.dma_start(ident[:], ident_d.ap())

        # ---- lbs loads: strip A -> partitions 0-47 (sync ring / even DMA
        # engines), strip B -> partitions 64-111 (scalar ring / odd
        # engines); 4 column chunks each so early chunks land fast ----
        lbs_sb = big.tile([112, TH], f16)
        CHN = 4
        cw = TH // CHN
        for c in range(CHN):
            c0, c1 = c * cw, (c + 1) * cw
            nc.sync.dma_start(
                lbs_sb[0:48, c0:c1], lbs_d.ap()[0:48, c0:c1]
            )
            nc.scalar.dma_start(
                lbs_sb[64:112, c0:c1], lbs_d.ap()[48:96, c0:c1]
            )

        # ---- stage 1: local in column layout, transpose, scatter ----
        pscol = psO.tile([Z, NWT], f32, tag="e0")
        for j in range(NWT):
            nc.tensor.matmul(
                pscol[:, j:j + 1], wt_sb[:, j * Z:(j + 1) * Z], g_sb[:],
                start=True, stop=True,
            )
        localcol = const.tile([Z, NWT], f16)
        nc.vector.tensor_add(localcol[:], pscol[:], biasc_sb[:])
        lT_ps = psO.tile([NWT, Z], f16, tag="e1")
        nc.tensor.transpose(lT_ps[:], localcol[:], ident[:])
        lT_sb = const.tile([NWT, Z], f16)
        nc.vector.tensor_copy(lT_sb[:], lT_ps[:])
        # DRAM roundtrip on the gpsimd (SWDGE) queue: re-partition into
        # the [24, 64] blocks of blkdiag(local, local) at bases 0 and 64.
        scratch = dram.tile([NWT, Z], f16)
        nc.gpsimd.dma_start(scratch[:], lT_sb[:])
        loc24 = scratch[:].rearrange("n (h f) -> (n h) f", h=2)
        lhsA = const.tile([48, 128], f16)
        lhsBt = const.tile([112, 128], f16)
        nc.vector.memset(lhsA[:], 0.0)
        nc.vector.memset(lhsBt[:], 0.0)
        nc.gpsimd.dma_start(lhsA[0:24, 0:64], loc24)
        nc.gpsimd.dma_start(lhsA[24:48, 64:128], loc24)
        nc.gpsimd.dma_start(lhsBt[64:88, 0:64], loc24)
        nc.gpsimd.dma_start(lhsBt[88:112, 64:128], loc24)
        lhsB = lhsBt[64:112, :]

        # ---- main loop: 16 tiles of [128, 1024], 2 matmuls each ----
        out_sb = big.tile([128, T // 2], f16)
        for jp in range(8):          # 1024-column group within strip
            for h in range(2):       # strip A / B
                idx = jp * 2 + h
                ps = psO.tile([128, 1024], f32, tag=f"e{idx % 2}")
                lhs = lhsA[:] if h == 0 else lhsB
                rlo = 0 if h == 0 else 64
                rhs = lbs_sb[rlo:rlo + 48, :]
                cb = jp * 1024
                nc.tensor.matmul(
                    ps[:, 0:512], lhs, rhs[:, cb:cb + 512],
                    start=True, stop=True, tile_position=(rlo, 0),
                )
                nc.tensor.matmul(
                    ps[:, 512:1024], lhs, rhs[:, cb + 512:cb + 1024],
                    start=True, stop=True, tile_position=(rlo, 0),
                )
                dcol = h * TH + cb
                if idx % 2 == 0:
                    nc.vector.tensor_copy(out_sb[:, dcol:dcol + 1024], ps[:])
                else:
                    nc.scalar.copy(out_sb[:, dcol:dcol + 1024], ps[:])
                nc.sync.dma_start(
                    out_d.ap()[:, dcol:dcol + 1024],
                    out_sb[:, dcol:dcol + 1024],
                )

    nc.compile()
    _built["nc"] = nc
    return nc


def make_in_maps(inputs):
    g_full = np.concatenate(
        [inputs["shape_code"], inputs["structure_code"], inputs["pose_code"]],
        axis=-1,
    ).astype(np.float16)  # (8, 128)
    # Wt[z, k*64+f] = W[k, f, z]
    wt = np.ascontiguousarray(
        inputs["W"].astype(np.float16).transpose(2, 0, 1).reshape(Z, KF)
    )
    # biasc[p, j] = bias_flat[j*128 + p]
    biasc = np.ascontiguousarray(
        inputs["bias"].astype(np.float16).reshape(NWT, Z).T
    )
    lbs = inputs["lbs_weights"].astype(np.float16)  # (B, T, K)
    in_maps = []
    for b in range(B):
        lb = np.ascontiguousarray(
            lbs[b].reshape(4, TH, K).transpose(0, 2, 1).reshape(4 * K, TH)
        )
        in_maps.append(
            {
                "lbs": lb,
                "wt": wt,
                "g": np.ascontiguousarray(g_full[b].reshape(Z, 1)),
                "biasc": biasc,
                "ident": np.eye(Z, dtype=np.float16),
            }
        )
    return in_maps


LAST_RESULT = None


def kernel(**inputs) -> np.ndarray:
    global LAST_RESULT
    nc = _build()
    in_maps = make_in_maps(inputs)
    res = bass_utils.run_bass_kernel_spmd(
        nc,
        in_maps,
        core_ids=list(range(B)),
        trace=os.environ.get("LFE_TRACE", "0") == "1",
    )
    LAST_RESULT = res
    out = np.empty((B, T, F), dtype=np.float32)
    for b in range(B):
        o = np.asarray(res.results[b]["out"])  # (128, 16384) fp16
        for h in range(2):
            cols = slice(h * TH, (h + 1) * TH)
            out[b, h * 2 * TH:h * 2 * TH + TH] = o[0:64, cols].T
            out[b, h * 2 * TH + TH:(h + 1) * 2 * TH] = o[64:128, cols].T
    return out


if __name__ == "__main__":
    rng = np.random.default_rng(0)
    inputs = {
        "shape_code": rng.standard_normal((B, 64), dtype=np.float32),
        "structure_code": rng.standard_normal((B, 32), dtype=np.float32),
        "pose_code": rng.standard_normal((B, 32), dtype=np.float32),
        "lbs_weights": rng.random((B, T, K), dtype=np.float32),
        "W": rng.standard_normal((K, F, Z), dtype=np.float32),
        "bias": rng.standard_normal((K, F), dtype=np.float32),
    }
    out = kernel(**inputs)
    g = np.concatenate(
        [inputs["shape_code"], inputs["structure_code"], inputs["pose_code"]], -1
    )
    local = np.einsum("kfz,bz->bkf", inputs["W"], g) + inputs["bias"][None]
    ref = np.einsum("btk,bkf->btf", inputs["lbs_weights"], local)
    err = np.abs(out - ref).max() / np.abs(ref).max()
    print("rel err:", err)


# revision 19
# speedup vs baseline: 1.3697x; 1.3697x over previous
"""Trainium2 Bass kernel for nn_LocalFeatureEncoder (fp16, no-DMA stage 1).

Computes, for B=8 batches on 8 NeuronCores (batch b -> core b):
    g      = concat(shape_code, structure_code, pose_code)      # (B, 128)
    local  = einsum('kfz,bz->bkf', W, g) + bias                 # (B, 24, 64)
    out    = einsum('btk,bkf->btf', lbs_weights, local)         # (B, 32768, 64)

All device data is fp16 (f32 PSUM accumulation). Per core: 0.55 MB const
blob + 1.57 MB lbs in, 4.19 MB out.

Stage 1 runs entirely on compute engines (no mid-chain DMAs, whose
completions straggle multiple us behind bulk traffic):
  12 matmuls (Wt blocks stationary, g moving) -> local column layout
  [128, 12]; DVE bias-add (fp16); PE-transpose -> lT [12, 128]; then 4
  selector matmuls (0/1 host consts) re-partition lT into
  blkdiag(local, local) at partition bases 0 and 64 in ONE psum tile
  [112, 128]; one DVE copy -> SBUF stationaries.
All small constants ship in ONE consolidated [128, 2128] fp16 DMA (bias
f32 bytes packed as fp16 pairs, bitcast back on device).

Main: lbs host-pretransposed, k-on-partition: strip A rows 0-47
 = (t 0..8191 | 8192..16383) stacked 2x24, strip B rows 64-111 same for
t 16384..32767. 16 psum tiles [128, 1024]; 2 matmuls N=512 each; A/B on
disjoint PE row groups, weights resident. Evacuation casts alternate
Vector/Scalar into out_sb; each 1024-col slab goes out as its own DMA,
alternating the two HWDGE rings behind that ring's input stream.
"""

import os
from contextlib import ExitStack

import numpy as np

import concourse.bass as bass
import concourse.bacc as bacc
import concourse.tile as tile
from concourse import mybir
from concourse import bass_utils

B, T, K, Z, F = 8, 32768, 24, 128, 64
KF = K * F              # 1536
TH = T // 4             # 8192 columns per strip
NWT = KF // Z           # 12

# const blob column offsets (fp16 columns; C_BIAS must be even so the
# f32 bitcast of the packed bias bytes stays 4-byte aligned)
C_WT = 0                # [128, 1536]
C_G = KF                # [128, 1]
C_BIAS = KF + 2         # [128, 24] = 12 f32 columns bit-packed
C_ID = KF + 26          # [128, 128]
C_SEL = KF + 154        # 4 x [128, 112] (rows 12+ zero)
C_TOT = KF + 154 + 4 * 112  # 2138 -> pad to 2144

_built = {}


def _build():
    if "nc" in _built:
        return _built["nc"]

    f32 = mybir.dt.float32
    f16 = mybir.dt.float16
    nc = bacc.Bacc("TRN2", target_bir_lowering=False, debug=False)

    CP = 2144
    cst_d = nc.dram_tensor("cst", (Z, CP), f16, kind="ExternalInput")
    lbs_d = nc.dram_tensor("lbs", (4 * K, TH), f16, kind="ExternalInput")
    out_d = nc.dram_tensor("out", (128, T // 2), f16, kind="ExternalOutput")

    with tile.TileContext(nc) as tc, ExitStack() as ctx:
        const = ctx.enter_context(tc.tile_pool(name="const", bufs=1))
        big = ctx.enter_context(tc.tile_pool(name="big", bufs=1))
        psO = ctx.enter_context(
            tc.tile_pool(name="psO", bufs=2, space=bass.MemorySpace.PSUM)
        )

        # ---- one consolidated const DMA, then lbs chunks ----
        cst = const.tile([Z, CP], f16)
        nc.sync.dma_start(cst[:], cst_d.ap())

        lbs_sb = big.tile([112, TH], f16)
        CHN = 4
        cw = TH // CHN
        for c in range(CHN):
            c0, c1 = c * cw, (c + 1) * cw
            nc.sync.dma_start(lbs_sb[0:48, c0:c1], lbs_d.ap()[0:48, c0:c1])
            nc.scalar.dma_start(
                lbs_sb[64:112, c0:c1], lbs_d.ap()[48:96, c0:c1]
            )

        wt_sb = cst[:, C_WT:C_WT + KF]
        g_sb = cst[:, C_G:C_G + 1]
        biasc = cst[:, C_BIAS:C_BIAS + 24].bitcast(f32)  # [128, 12] f32
        ident = cst[:, C_ID:C_ID + Z]

        # ---- stage 1 ----
        pscol = psO.tile([Z, NWT], f32, tag="e0")
        for j in range(NWT):
            nc.tensor.matmul(
                pscol[:, j:j + 1], wt_sb[:, j * Z:(j + 1) * Z], g_sb,
                start=True, stop=True,
            )
        localcol = const.tile([Z, NWT], f16)
        nc.vector.tensor_add(localcol[:], pscol[:], biasc)
        lT_ps = psO.tile([NWT, Z], f16, tag="e1")
        nc.tensor.transpose(lT_ps[:], localcol[:], ident)
        lT_sb = const.tile([NWT, Z], f16)
        nc.vector.tensor_copy(lT_sb[:], lT_ps[:])

        # selector matmuls: psX[c, m] = blkdiag(local, local) rows at
        # partition bases 0 (strip A) and 64 (strip B).
        psX = psO.tile([112, Z], f32, tag="e0")
        starts = [True, False, False, False]
        stops = [False, False, False, True]
        for i in range(4):
            sel = cst[0:NWT, C_SEL + i * 112:C_SEL + (i + 1) * 112]
            rhs = lT_sb[:, (i % 2) * 64:(i % 2) * 64 + 64]
            outc = (i // 2) * 64
            nc.tensor.matmul(
                psX[:, outc:outc + 64], sel, rhs,
                start=starts[i], stop=stops[i],
            )
        lhs_both = const.tile([112, Z], f16)
        nc.vector.tensor_copy(lhs_both[:], psX[:])
        lhsA = lhs_both[0:48, :]
        lhsB = lhs_both[64:112, :]

        # ---- main loop: 16 tiles of [128, 1024], 2 matmuls each ----
        out_sb = big.tile([128, T // 2], f16)
        for jp in range(8):
            for h in range(2):
                idx = jp * 2 + h
                ps = psO.tile([128, 1024], f32, tag=f"e{idx % 2}")
                lhs = lhsA if h == 0 else lhsB
                rlo = 0 if h == 0 else 64
                rhs = lbs_sb[rlo:rlo + 48, :]
                cb = jp * 1024
                nc.tensor.matmul(
                    ps[:, 0:512], lhs, rhs[:, cb:cb + 512],
                    start=True, stop=True, tile_position=(rlo, 0),
                )
                nc.tensor.matmul(
                    ps[:, 512:1024], lhs, rhs[:, cb + 512:cb + 1024],
                    start=True, stop=True, tile_position=(rlo, 0),
                )
                dcol = h * TH + cb
                if idx % 2 == 0:
                    nc.vector.tensor_copy(out_sb[:, dcol:dcol + 1024], ps[:])
                else:
                    nc.scalar.copy(out_sb[:, dcol:dcol + 1024], ps[:])
                eng = nc.sync if idx % 2 == 0 else nc.scalar
                eng.dma_start(
                    out_d.ap()[:, dcol:dcol + 1024],
                    out_sb[:, dcol:dcol + 1024],
                )

    nc.compile()
    _built["nc"] = nc
    return nc


def _make_cst(inputs, gb):
    cst = np.zeros((Z, 2144), np.float16)
    cst[:, C_WT:C_WT + KF] = (
        inputs["W"].astype(np.float16).transpose(2, 0, 1).reshape(Z, KF)
    )
    cst[:, C_G] = gb
    biasc = inputs["bias"].astype(np.float32).reshape(NWT, Z).T  # [128, 12]
    cst[:, C_BIAS:C_BIAS + 24] = np.ascontiguousarray(biasc).view(np.float16)
    cst[:, C_ID:C_ID + Z] = np.eye(Z, dtype=np.float16)
    # selector i: psX[c, m-block i//2] += sel_i^T @ lT[:, (i%2)*64:+64]
    # nonzero where local k-row = c' (c' = c or c-64), parity matches.
    for i in range(4):
        sel = np.zeros((Z, 112), np.float16)
        par = i % 2          # lT column half = k parity
        blk = i // 2         # output f-block = blkdiag block = c' // 24
        for base in (0, 64):
            for k in range(K):
                c = base + blk * K + k
                if k % 2 == par and c < base + 48:
                    sel[k // 2, c] = 1.0
        cst[:, C_SEL + i * 112:C_SEL + (i + 1) * 112] = sel
    return cst


def make_in_maps(inputs):
    g_full = np.concatenate(
        [inputs["shape_code"], inputs["structure_code"], inputs["pose_code"]],
        axis=-1,
    ).astype(np.float16)  # (8, 128)
    lbs = inputs["lbs_weights"].astype(np.float16)  # (B, T, K)
    in_maps = []
    for b in range(B):
        lb = np.ascontiguousarray(
            lbs[b].reshape(4, TH, K).transpose(0, 2, 1).reshape(4 * K, TH)
        )
        in_maps.append({"lbs": lb, "cst": _make_cst(inputs, g_full[b])})
    return in_maps


LAST_RESULT = None


def kernel(**inputs) -> np.ndarray:
    global LAST_RESULT
    nc = _build()
    in_maps = make_in_maps(inputs)
    res = bass_utils.run_bass_kernel_spmd(
        nc,
        in_maps,
        core_ids=list(range(B)),
        trace=os.environ.get("LFE_TRACE", "0") == "1",
    )
    LAST_RESULT = res
    out = np.empty((B, T, F), dtype=np.float32)
    for b in range(B):
        o = np.asarray(res.results[b]["out"])  # (128, 16384) fp16
        for h in range(2):
            cols = slice(h * TH, (h + 1) * TH)
            out[b, h * 2 * TH:h * 2 * TH + TH] = o[0:64, cols].T
            out[b, h * 2 * TH + TH:(h + 1) * 2 * TH] = o[64:128, cols].T
    return out


if __name__ == "__main__":
    rng = np.random.default_rng(0)
    inputs = {
        "shape_code": rng.standard_normal((B, 64), dtype=np.float32),
        "structure_code": rng.standard_normal((B, 32), dtype=np.float32),
        "pose_code": rng.standard_normal((B, 32), dtype=np.float32),
        "lbs_weights": rng.random((B, T, K), dtype=np.float32),
        "W": rng.standard_normal((K, F, Z), dtype=np.float32),
        "bias": rng.standard_normal((K, F), dtype=np.float32),
    }
    out = kernel(**inputs)
    g = np.concatenate(
        [inputs["shape_code"], inputs["structure_code"], inputs["pose_code"]], -1
    )
    local = np.einsum("kfz,bz->bkf", inputs["W"], g) + inputs["bias"][None]
    ref = np.einsum("btk,bkf->btf", inputs["lbs_weights"], local)
    err = np.abs(out - ref).max() / np.abs(ref).max()
    print("rel err:", err)
